# revision 33
# baseline (speedup 1.0000x reference)
"""GPTQ 4-bit quantized linear on 8 Trainium2 NeuronCores.

y[b,s,o] = sum_i x[b,s,i] * W[o,i] + bias[o]
  W[o,i] = (nib(qweight)[o,i] - zeros[o,i//128]) * scales[o,i//128]

Sharding: 4-way over out_features x 2-way over tokens (8 cores).
Per core: out shard [4096 tokens, 1024 outs].

Strategy (v6): mixed-precision split-K with a group-mean sidecar.
  - k-tiles 0..NF-1 (NF=16) run as e4m3 DoubleRow matmuls (2 k-tiles
    per MM, ~1.93x the bf16/fp16 rate).  Before fp8 quantization the
    per-group mean of W is subtracted (the GPTQ zero/scale structure
    puts ~half of W's energy in the per-group mean), halving both the
    W-quant and x-quant error energy of the fp8 part.
  - The subtracted mean term y += groupsum(x)[t,g] * m[o,g] is exact
    and rank-NF; it is restored on the host during unshard (fp32, like
    the bias add), so the device runs only the 32 real k-tiles.
  - k-tiles NF..31 run in fp16 (PE upcasts to e10m11, so fp16 keeps 10
    mantissa bits at bf16 speed - lower error than bf16 for free).
  - Output stays fp32 (skips the bf16 drain rounding).
  - A per-out-row scale (chosen per row by grid search to minimize fp8
    W error) is divided out of ALL W parts on host and multiplied back
    into the output on host, so PSUM accumulation stays consistent.
  - Phase A: chunks 0+1 k-unit-synchronized on all 8 PSUM banks so the
    resident W load hides under compute.  Phase B: chunks 2-15
    tsub-outer / unit-inner with W fully resident.
"""

from contextlib import ExitStack

import numpy as np
import ml_dtypes

import concourse.bass as bass
import concourse.mybir as mybir
import concourse.tile as tile
from concourse.bass_utils import run_bass_kernel_spmd

F32 = mybir.dt.float32
FP16 = mybir.dt.float16
E4M3 = mybir.dt.float8e4
DR = mybir.MatmulPerfMode.DoubleRow

# Problem shape (hardcoded; kernel.py must be self-contained).
B, S, IN, OUT = 4, 2048, 4096, 4096
TOK = B * S
GROUP = 128
O_WAYS, T_WAYS = 4, 2
N_CORES = 8

TSH = TOK // T_WAYS      # tokens per core (4096)
OSH = OUT // O_WAYS      # out features per core (1024)
NK = IN // 128           # k tiles (32)
CHUNK = 256              # tokens per chunk
N_CHUNK = TSH // CHUNK   # 16
N_TSUB = CHUNK // 128    # 2
RHS_W = 512
N_RHS = OSH // RHS_W     # 2

NF = 16                  # fp8 k-tiles (must be even)
NPAIR = NF // 2          # DoubleRow pairs (8)
KF = NF * 128
NB = NK - NF             # fp16 k-tiles (16)
NBV = NB                 # fp16 tiles on device (virtual tile moved to host)

F8 = ml_dtypes.float8_e4m3  # IEEE e4m3 (inf at 0x78, max 240) == TRN FP8_EXP4


def build_nc():
    nc = bass.Bass()
    xt8_d = nc.declare_dram_parameter(
        "xt8", [N_CHUNK, 128, 2, NPAIR * CHUNK], E4M3, isOutput=False
    )
    xtb_d = nc.declare_dram_parameter(
        "xtb", [N_CHUNK, 128, NBV * CHUNK], FP16, isOutput=False
    )
    wt8_d = nc.declare_dram_parameter("wt8", [NPAIR, 128, 2, OSH], E4M3, isOutput=False)
    wtb_d = nc.declare_dram_parameter("wtb", [NBV, 128, OSH], FP16, isOutput=False)
    out_d = nc.declare_dram_parameter("out", [TSH, OSH], F32, isOutput=True)

    with tile.TileContext(nc) as tc, ExitStack() as ctx:
        P = 128
        pool_wt = ctx.enter_context(tc.tile_pool(name="wt", bufs=1))
        pool_x = ctx.enter_context(tc.tile_pool(name="x", bufs=3))
        pool_ob = ctx.enter_context(tc.tile_pool(name="ob", bufs=4))
        psum_mm = ctx.enter_context(tc.tile_pool(name="psm", bufs=8, space="PSUM"))

        # ---- W load: fp8 pair tiles then fp16 tiles, two queues ----
        WT8, WTB = [], []
        qi = 0
        for p in range(NPAIR):
            wt = pool_wt.tile([P, 2, OSH], E4M3, tag=f"wt8{p}", name=f"wt8{p}")
            eng = nc.scalar if qi % 2 == 0 else nc.gpsimd
            eng.dma_start(out=wt[:], in_=wt8_d[p, :, :, :])
            WT8.append(wt)
            qi += 1
        for j in range(NBV):
            wt = pool_wt.tile([P, OSH], FP16, tag=f"wtb{j}", name=f"wtb{j}")
            eng = nc.scalar if qi % 2 == 0 else nc.gpsimd
            eng.dma_start(out=wt[:], in_=wtb_d[j, :, :])
            WTB.append(wt)
            qi += 1

        def make_ps(ch):
            return [
                [
                    psum_mm.tile([P, RHS_W], F32, tag="ps", name=f"ps{ch}_{t}_{r}")
                    for r in range(N_RHS)
                ]
                for t in range(N_TSUB)
            ]

        def drain(ch, tsub, ps, nsplit=2):
            ob = pool_ob.tile([P, OSH], F32, tag="ob", name=f"ob{ch}_{tsub}")
            t0 = ch * CHUNK + tsub * P
            w = OSH // nsplit
            for j in range(nsplit):
                nc.vector.tensor_copy(
                    ob[:, j * w : (j + 1) * w],
                    ps[tsub][(j * w) // RHS_W][:, (j * w) % RHS_W : (j * w) % RHS_W + w],
                )
                eng = nc.scalar if j % 2 == 0 else nc.sync
                eng.dma_start(
                    out=out_d[t0 : t0 + P, j * w : (j + 1) * w],
                    in_=ob[:, j * w : (j + 1) * w],
                )

        def mm8(ps_r, xc8, p, tsub, r, start):
            t0 = p * CHUNK + tsub * P
            nc.tensor.matmul(
                ps_r[:],
                xc8[:, 0:2, t0 : t0 + P],
                WT8[p][:, 0:2, r * RHS_W : (r + 1) * RHS_W],
                start=start,
                stop=False,
                perf_mode=DR,
            )

        def mmb(ps_r, xcb, j, tsub, r, stop):
            t0 = j * CHUNK + tsub * P
            nc.tensor.matmul(
                ps_r[:],
                xcb[:, t0 : t0 + P],
                WTB[j][:, r * RHS_W : (r + 1) * RHS_W],
                start=False,
                stop=stop,
            )

        # ---- phase A: chunks 0+1 unit-synchronized on all 8 PSUM banks ----
        xc8s, xcbs = {}, {}
        for ch in range(2):
            xc8s[ch] = pool_x.tile([P, 2, NPAIR * CHUNK], E4M3, tag="xc8", name=f"xc8_{ch}")
            xcbs[ch] = pool_x.tile([P, NBV * CHUNK], FP16, tag="xcb", name=f"xcb_{ch}")
        for ch in range(2):
            nc.sync.dma_start(out=xc8s[ch][:], in_=xt8_d[ch, :, :, :])
        o = 0
        for n in (6, 5, 5):
            for ch in range(2):
                nc.sync.dma_start(
                    out=xcbs[ch][:, o * CHUNK : (o + n) * CHUNK],
                    in_=xtb_d[ch, :, o * CHUNK : (o + n) * CHUNK],
                )
            o += n
        psA = {ch: make_ps(ch) for ch in range(2)}
        # chunk-1's x lands a few us after chunk-0's; defer the first two
        # units' chunk-1 matmuls so chunk-0 work fills that window.
        sched = [(0, 0), (1, 0), (2, 0), (3, 0), (0, 1), (1, 1), (2, 1), (3, 1)]
        sched += [(p, ch) for p in range(4, NPAIR) for ch in range(2)]
        for p, ch in sched:
            for tsub in range(N_TSUB):
                for r in range(N_RHS):
                    mm8(psA[ch][tsub][r], xc8s[ch], p, tsub, r, start=(p == 0))
        for j in range(NBV):
            for ch in range(2):
                for tsub in range(N_TSUB):
                    for r in range(N_RHS):
                        mmb(psA[ch][tsub][r], xcbs[ch], j, tsub, r, stop=(j == NBV - 1))
        for ch in range(2):
            for tsub in range(N_TSUB):
                drain(ch, tsub, psA[ch])

        # ---- phase B: chunks 2-15, tsub-outer / unit-inner ----
        for ch in range(2, N_CHUNK):
            xc8 = pool_x.tile([P, 2, NPAIR * CHUNK], E4M3, tag="xc8", name=f"xc8_{ch}")
            xcb = pool_x.tile([P, NBV * CHUNK], FP16, tag="xcb", name=f"xcb_{ch}")
            nc.gpsimd.dma_start(out=xc8[:], in_=xt8_d[ch, :, :, :])
            nc.sync.dma_start(out=xcb[:], in_=xtb_d[ch, :, :])
            ps = make_ps(ch)
            for tsub in range(N_TSUB):
                last = ch == N_CHUNK - 1 and tsub == N_TSUB - 1
                if not last:
                    for p in range(NPAIR):
                        for r in range(N_RHS):
                            mm8(ps[tsub][r], xc8, p, tsub, r, start=(p == 0))
                    for j in range(NBV):
                        for r in range(N_RHS):
                            mmb(ps[tsub][r], xcb, j, tsub, r, stop=(j == NBV - 1))
                    drain(ch, tsub, ps)
                    continue
                # final tile: separate r-sweeps so r=0's drain+store
                # overlap r=1's matmuls; r=1 drains/stores in quarters.
                ob = pool_ob.tile([P, OSH], F32, tag="ob", name=f"ob{ch}_{tsub}")
                t0 = ch * CHUNK + tsub * P
                for p in range(NPAIR):
                    mm8(ps[tsub][0], xc8, p, tsub, 0, start=(p == 0))
                for j in range(NBV):
                    mmb(ps[tsub][0], xcb, j, tsub, 0, stop=(j == NBV - 1))
                nc.vector.tensor_copy(ob[:, 0:RHS_W], ps[tsub][0][:])
                nc.scalar.dma_start(
                    out=out_d[t0 : t0 + P, 0:RHS_W], in_=ob[:, 0:RHS_W]
                )
                # r=1 as two quarter-bank chains: the first quarter's
                # drain+store overlap the second quarter's matmuls.
                # Full-bank tiles so each group owns its 2KB zero region.
                QW = RHS_W // 2
                psq = [
                    psum_mm.tile([P, RHS_W], F32, tag="ps", name=f"psq{q}")
                    for q in range(2)
                ]
                ts0 = tsub * P
                for q in range(2):
                    c0 = RHS_W + q * QW
                    for p in range(NPAIR):
                        nc.tensor.matmul(
                            psq[q][:, 0:QW],
                            xc8[:, 0:2, p * CHUNK + ts0 : p * CHUNK + ts0 + P],
                            WT8[p][:, 0:2, c0 : c0 + QW],
                            start=(p == 0),
                            stop=False,
                            perf_mode=DR,
                        )
                    for j in range(NBV):
                        nc.tensor.matmul(
                            psq[q][:, 0:QW],
                            xcb[:, j * CHUNK + ts0 : j * CHUNK + ts0 + P],
                            WTB[j][:, c0 : c0 + QW],
                            start=False,
                            stop=(j == NBV - 1),
                        )
                    nc.vector.tensor_copy(ob[:, c0 : c0 + QW], psq[q][:, 0:QW])
                    if q == 0:
                        nc.sync.dma_start(
                            out=out_d[t0 : t0 + P, c0 : c0 + QW],
                            in_=ob[:, c0 : c0 + QW],
                        )
                    else:
                        h = QW // 2
                        nc.scalar.dma_start(
                            out=out_d[t0 : t0 + P, c0 : c0 + h],
                            in_=ob[:, c0 : c0 + h],
                        )
                        nc.sync.dma_start(
                            out=out_d[t0 : t0 + P, c0 + h : c0 + QW],
                            in_=ob[:, c0 + h : c0 + QW],
                        )
    _legalize_waits(nc)
    return nc


_SPLIT_TYPES = (
    "InstTensorTensor",
    "InstTensorScalarPtr",
    "InstTensorScalar",
    "InstActivation",
    "InstTensorCopy",
    "InstMatmult",
    "InstDMACopy",
    "InstDrain",
)


def _legalize_waits(nc):
    """walrus allows only one on-inst sync wait for DVE/ACT elementwise
    instruction encodings; split extra waits onto same-engine Drains."""
    f = nc.m.functions[0]
    n = 0
    for blk in f.blocks:
        out_insts = []
        for inst in blk.instructions:
            si = inst.sync_info
            if (
                si is not None
                and len(si.on_wait) > 1
                and type(inst).__name__ in _SPLIT_TYPES
            ):
                waits = list(si.on_wait)
                for w in waits[:-1]:
                    d = mybir.InstDrain(name=f"waitfix{n}", ins=[], outs=[])
                    d.engine = inst.engine
                    d.sync_info = mybir.SyncInfo(on_wait=[w], on_update=[])
                    out_insts.append(d)
                    n += 1
                inst.sync_info = mybir.SyncInfo(
                    on_wait=[waits[-1]], on_update=list(si.on_update)
                )
            out_insts.append(inst)
        blk.instructions = out_insts


_NC_CACHE = {}


def _get_nc(key=()):
    if key not in _NC_CACHE:
        _NC_CACHE[key] = build_nc()
    return _NC_CACHE[key]


def _quant_w_fp8(Wp):
    """Per-row scale search: minimize fp8 rounding error of each row.
    Returns (raw fp8 values of Wp/s_row, s_row)."""
    rowmax = np.abs(Wp).max(axis=1, keepdims=True)
    cands = np.geomspace(0.55, 1.05, 24)
    best_q = np.empty(Wp.shape, dtype=F8)
    best_e = np.full(Wp.shape[0], np.inf)
    best_s = np.empty((Wp.shape[0], 1), np.float32)
    for c in cands:
        s_row = (rowmax / (240.0 * c)).astype(np.float32)
        q = np.clip(Wp / s_row, -240, 240).astype(F8)
        e2 = ((q.astype(np.float32) * s_row - Wp) ** 2).sum(axis=1)
        take = e2 < best_e
        best_q[take] = q[take]
        best_e[take] = e2[take]
        best_s[take] = s_row[take]
    return best_q, best_s[:, 0]


def make_in_maps(x, qweight, scales, zeros, bias):
    x2 = np.asarray(x).reshape(TOK, IN)
    qweight = np.asarray(qweight)
    scales = np.asarray(scales).astype(np.float32)
    zeros = np.asarray(zeros).astype(np.float32)

    # ---- dequantize W on host ----
    nib = np.empty((OUT, IN), np.float32)
    nib[:, 0::2] = (qweight & 15).astype(np.float32)
    nib[:, 1::2] = ((qweight >> 4) & 15).astype(np.float32)
    W = (nib.reshape(OUT, NK, GROUP) - zeros[:, :, None]) * scales[:, :, None]
    W = W.reshape(OUT, IN)

    # ---- fp8 part: group-mean removal + per-row scale search ----
    Wf = W[:, :KF].reshape(OUT, NF, GROUP)
    m = Wf.mean(axis=2)                                   # [OUT, NF]
    Wp = (Wf - m[:, :, None]).reshape(OUT, KF)
    Wq8, s_row = _quant_w_fp8(Wp)                         # raw fp8, [OUT]

    inv_s = (1.0 / s_row)[:, None].astype(np.float32)
    Wb = (W[:, KF:] * inv_s).astype(np.float16)           # [OUT, KB] fp16

    # ---- x: fp8 part (DoubleRow pair layout) + fp16 part + virtual ----
    xg_full = x2[:, :KF].reshape(TOK, NF, GROUP).sum(axis=2)  # fp32 group sums

    xt8_shards, xtb_shards = [], []
    for t in range(T_WAYS):
        xs = x2[t * TSH : (t + 1) * TSH]
        xq8 = xs[:, :KF].astype(F8)                       # [TSH, KF] raw fp8
        a8 = np.ascontiguousarray(xq8.T)                  # [KF, TSH]
        a8 = a8.reshape(NPAIR, 2, 128, N_CHUNK, CHUNK).transpose(3, 2, 1, 0, 4)
        xt8_shards.append(
            np.ascontiguousarray(a8.reshape(N_CHUNK, 128, 2, NPAIR * CHUNK))
        )

        xb = xs[:, KF:].astype(np.float16)                # [TSH, KB]
        ab = np.ascontiguousarray(xb.T)
        ab = ab.reshape(NB, 128, N_CHUNK, CHUNK).transpose(2, 1, 0, 3)
        xtb_shards.append(
            np.ascontiguousarray(ab.reshape(N_CHUNK, 128, NBV * CHUNK))
        )

    in_maps = []
    srow_shards = []
    for c in range(N_CORES):
        o0 = (c % O_WAYS) * OSH
        wq8 = Wq8[o0 : o0 + OSH]                          # [OSH, KF] fp8
        w8 = np.ascontiguousarray(wq8.T)                  # [KF, OSH]
        w8 = np.ascontiguousarray(
            w8.reshape(NPAIR, 2, 128, OSH).transpose(0, 2, 1, 3)
        )                                                 # [NPAIR, 128, 2, OSH]

        wtb = np.ascontiguousarray(Wb[o0 : o0 + OSH].T).reshape(NB, 128, OSH)

        in_maps.append(
            {
                "xt8": xt8_shards[c // O_WAYS],
                "xtb": xtb_shards[c // O_WAYS],
                "wt8": w8,
                "wtb": wtb,
            }
        )
        srow_shards.append(s_row[o0 : o0 + OSH])
    return in_maps, srow_shards, xg_full, m


def _run(x, qweight, scales, zeros, bias, trace=False, **kw):
    nc = _get_nc()
    in_maps, srow_shards, xg_full, m = make_in_maps(x, qweight, scales, zeros, bias)
    res = run_bass_kernel_spmd(nc, in_maps, list(range(N_CORES)), trace=trace, **kw)
    full = np.empty((TOK, OUT), dtype=np.float32)
    for c in range(N_CORES):
        o0 = (c % O_WAYS) * OSH
        t0 = (c // O_WAYS) * TSH
        full[t0 : t0 + TSH, o0 : o0 + OSH] = np.asarray(res.results[c]["out"]).astype(
            np.float32
        ) * srow_shards[c][None, :]
    # exact rank-NF group-mean term (removed from W before fp8 quantization)
    full += xg_full @ m.T
    full += np.asarray(bias, dtype=np.float32)[None, :]
    return full.reshape(B, S, OUT), res


def kernel(x, qweight, scales, zeros, bias):
    out, _ = _run(x, qweight, scales, zeros, bias)
    return out


# revision 34
# speedup vs baseline: 1.0046x; 1.0046x over previous
"""GPTQ 4-bit quantized linear on 8 Trainium2 NeuronCores.

y[b,s,o] = sum_i x[b,s,i] * W[o,i] + bias[o]
  W[o,i] = (nib(qweight)[o,i] - zeros[o,i//128]) * scales[o,i//128]

Sharding: 4-way over out_features x 2-way over tokens (8 cores).
Per core: out shard [4096 tokens, 1024 outs].

Strategy (v6): mixed-precision split-K with a group-mean sidecar.
  - k-tiles 0..NF-1 (NF=16) run as e4m3 DoubleRow matmuls (2 k-tiles
    per MM, ~1.93x the bf16/fp16 rate).  Before fp8 quantization the
    per-group mean of W is subtracted (the GPTQ zero/scale structure
    puts ~half of W's energy in the per-group mean), halving both the
    W-quant and x-quant error energy of the fp8 part.
  - The subtracted mean term y += groupsum(x)[t,g] * m[o,g] is exact
    and rank-NF; it is restored on the host during unshard (fp32, like
    the bias add), so the device runs only the 32 real k-tiles.
  - k-tiles NF..31 run in fp16 (PE upcasts to e10m11, so fp16 keeps 10
    mantissa bits at bf16 speed - lower error than bf16 for free).
  - Output stays fp32 (skips the bf16 drain rounding).
  - A per-out-row scale (chosen per row by grid search to minimize fp8
    W error) is divided out of ALL W parts on host and multiplied back
    into the output on host, so PSUM accumulation stays consistent.
  - Phase A: chunks 0+1 k-unit-synchronized on all 8 PSUM banks so the
    resident W load hides under compute.  Phase B: chunks 2-15
    tsub-outer / unit-inner with W fully resident.
"""

from contextlib import ExitStack

import numpy as np
import ml_dtypes

import concourse.bass as bass
import concourse.mybir as mybir
import concourse.tile as tile
from concourse.bass_utils import run_bass_kernel_spmd

F32 = mybir.dt.float32
FP16 = mybir.dt.float16
E4M3 = mybir.dt.float8e4
DR = mybir.MatmulPerfMode.DoubleRow

# Problem shape (hardcoded; kernel.py must be self-contained).
B, S, IN, OUT = 4, 2048, 4096, 4096
TOK = B * S
GROUP = 128
O_WAYS, T_WAYS = 4, 2
N_CORES = 8

TSH = TOK // T_WAYS      # tokens per core (4096)
OSH = OUT // O_WAYS      # out features per core (1024)
NK = IN // 128           # k tiles (32)
CHUNK = 256              # tokens per chunk
N_CHUNK = TSH // CHUNK   # 16
N_TSUB = CHUNK // 128    # 2
RHS_W = 512
N_RHS = OSH // RHS_W     # 2

NF = 16                  # fp8 k-tiles (must be even)
NPAIR = NF // 2          # DoubleRow pairs (8)
KF = NF * 128
NB = NK - NF             # fp16 k-tiles (16)
NBV = NB                 # fp16 tiles on device (virtual tile moved to host)

F8 = ml_dtypes.float8_e4m3  # IEEE e4m3 (inf at 0x78, max 240) == TRN FP8_EXP4


def build_nc():
    nc = bass.Bass()
    xt8_d = nc.declare_dram_parameter(
        "xt8", [N_CHUNK, 128, 2, NPAIR * CHUNK], E4M3, isOutput=False
    )
    xtb_d = nc.declare_dram_parameter(
        "xtb", [N_CHUNK, 128, NBV * CHUNK], FP16, isOutput=False
    )
    wt8_d = nc.declare_dram_parameter("wt8", [NPAIR, 128, 2, OSH], E4M3, isOutput=False)
    wtb_d = nc.declare_dram_parameter("wtb", [NBV, 128, OSH], FP16, isOutput=False)
    out_d = nc.declare_dram_parameter("out", [TSH, OSH], F32, isOutput=True)

    with tile.TileContext(nc) as tc, ExitStack() as ctx:
        P = 128
        pool_wt = ctx.enter_context(tc.tile_pool(name="wt", bufs=1))
        pool_x = ctx.enter_context(tc.tile_pool(name="x", bufs=3))
        pool_ob = ctx.enter_context(tc.tile_pool(name="ob", bufs=4))
        psum_mm = ctx.enter_context(tc.tile_pool(name="psm", bufs=8, space="PSUM"))

        # ---- W load: fp8 pair tiles then fp16 tiles, two queues ----
        WT8, WTB = [], []
        qi = 0
        for p in range(NPAIR):
            wt = pool_wt.tile([P, 2, OSH], E4M3, tag=f"wt8{p}", name=f"wt8{p}")
            eng = nc.scalar if qi % 2 == 0 else nc.gpsimd
            eng.dma_start(out=wt[:], in_=wt8_d[p, :, :, :])
            WT8.append(wt)
            qi += 1
        for j in range(NBV):
            wt = pool_wt.tile([P, OSH], FP16, tag=f"wtb{j}", name=f"wtb{j}")
            eng = nc.scalar if qi % 2 == 0 else nc.gpsimd
            eng.dma_start(out=wt[:], in_=wtb_d[j, :, :])
            WTB.append(wt)
            qi += 1

        def make_ps(ch):
            return [
                [
                    psum_mm.tile([P, RHS_W], F32, tag="ps", name=f"ps{ch}_{t}_{r}")
                    for r in range(N_RHS)
                ]
                for t in range(N_TSUB)
            ]

        def drain(ch, tsub, ps, nsplit=2):
            ob = pool_ob.tile([P, OSH], F32, tag="ob", name=f"ob{ch}_{tsub}")
            t0 = ch * CHUNK + tsub * P
            w = OSH // nsplit
            for j in range(nsplit):
                nc.vector.tensor_copy(
                    ob[:, j * w : (j + 1) * w],
                    ps[tsub][(j * w) // RHS_W][:, (j * w) % RHS_W : (j * w) % RHS_W + w],
                )
                eng = nc.scalar if j % 2 == 0 else nc.sync
                eng.dma_start(
                    out=out_d[t0 : t0 + P, j * w : (j + 1) * w],
                    in_=ob[:, j * w : (j + 1) * w],
                )

        def mm8(ps_r, xc8, p, tsub, r, start):
            t0 = p * CHUNK + tsub * P
            nc.tensor.matmul(
                ps_r[:],
                xc8[:, 0:2, t0 : t0 + P],
                WT8[p][:, 0:2, r * RHS_W : (r + 1) * RHS_W],
                start=start,
                stop=False,
                perf_mode=DR,
            )

        def mmb(ps_r, xcb, j, tsub, r, stop):
            t0 = j * CHUNK + tsub * P
            nc.tensor.matmul(
                ps_r[:],
                xcb[:, t0 : t0 + P],
                WTB[j][:, r * RHS_W : (r + 1) * RHS_W],
                start=False,
                stop=stop,
            )

        # ---- phase A: chunks 0+1 unit-synchronized on all 8 PSUM banks ----
        xc8s, xcbs = {}, {}
        for ch in range(2):
            xc8s[ch] = pool_x.tile([P, 2, NPAIR * CHUNK], E4M3, tag="xc8", name=f"xc8_{ch}")
            xcbs[ch] = pool_x.tile([P, NBV * CHUNK], FP16, tag="xcb", name=f"xcb_{ch}")
        for ch in range(2):
            nc.sync.dma_start(out=xc8s[ch][:], in_=xt8_d[ch, :, :, :])
        o = 0
        for n in (6, 5, 5):
            for ch in range(2):
                nc.sync.dma_start(
                    out=xcbs[ch][:, o * CHUNK : (o + n) * CHUNK],
                    in_=xtb_d[ch, :, o * CHUNK : (o + n) * CHUNK],
                )
            o += n
        psA = {ch: make_ps(ch) for ch in range(2)}
        # chunk-1's x lands a few us after chunk-0's; defer the first two
        # units' chunk-1 matmuls so chunk-0 work fills that window.
        sched = [(0, 0), (1, 0), (2, 0), (3, 0), (0, 1), (1, 1), (2, 1), (3, 1)]
        sched += [(p, ch) for p in range(4, NPAIR) for ch in range(2)]
        for p, ch in sched:
            for tsub in range(N_TSUB):
                for r in range(N_RHS):
                    mm8(psA[ch][tsub][r], xc8s[ch], p, tsub, r, start=(p == 0))
        for j in range(NBV):
            for ch in range(2):
                for tsub in range(N_TSUB):
                    for r in range(N_RHS):
                        mmb(psA[ch][tsub][r], xcbs[ch], j, tsub, r, stop=(j == NBV - 1))
        for ch in range(2):
            for tsub in range(N_TSUB):
                drain(ch, tsub, psA[ch])

        # ---- phase B: chunks 2-15, tsub-outer / unit-inner ----
        for ch in range(2, N_CHUNK):
            xc8 = pool_x.tile([P, 2, NPAIR * CHUNK], E4M3, tag="xc8", name=f"xc8_{ch}")
            xcb = pool_x.tile([P, NBV * CHUNK], FP16, tag="xcb", name=f"xcb_{ch}")
            nc.gpsimd.dma_start(out=xc8[:], in_=xt8_d[ch, :, :, :])
            nc.sync.dma_start(out=xcb[:], in_=xtb_d[ch, :, :])
            ps = make_ps(ch)
            for tsub in range(N_TSUB):
                last = ch == N_CHUNK - 1 and tsub == N_TSUB - 1
                if not last:
                    for p in range(NPAIR):
                        for r in range(N_RHS):
                            mm8(ps[tsub][r], xc8, p, tsub, r, start=(p == 0))
                    for j in range(NBV):
                        for r in range(N_RHS):
                            mmb(ps[tsub][r], xcb, j, tsub, r, stop=(j == NBV - 1))
                    drain(ch, tsub, ps)
                    continue
                # final tile: separate r-sweeps so r=0's drain+store
                # overlap r=1's matmuls; r=1 drains/stores in quarters.
                ob = pool_ob.tile([P, OSH], F32, tag="ob", name=f"ob{ch}_{tsub}")
                t0 = ch * CHUNK + tsub * P
                for r in range(N_RHS):
                    for p in range(NPAIR):
                        mm8(ps[tsub][r], xc8, p, tsub, r, start=(p == 0))
                    for j in range(NBV):
                        mmb(ps[tsub][r], xcb, j, tsub, r, stop=(j == NBV - 1))
                    nq = 1 if r == 0 else 4
                    w = RHS_W // nq
                    for j in range(nq):
                        c0 = r * RHS_W + j * w
                        nc.vector.tensor_copy(
                            ob[:, c0 : c0 + w],
                            ps[tsub][r][:, j * w : (j + 1) * w],
                        )
                        eng = nc.scalar if j % 2 == 0 else nc.sync
                        eng.dma_start(
                            out=out_d[t0 : t0 + P, c0 : c0 + w],
                            in_=ob[:, c0 : c0 + w],
                        )
    _legalize_waits(nc)
    return nc


_SPLIT_TYPES = (
    "InstTensorTensor",
    "InstTensorScalarPtr",
    "InstTensorScalar",
    "InstActivation",
    "InstTensorCopy",
    "InstMatmult",
    "InstDMACopy",
    "InstDrain",
)


def _legalize_waits(nc):
    """walrus allows only one on-inst sync wait for DVE/ACT elementwise
    instruction encodings; split extra waits onto same-engine Drains."""
    f = nc.m.functions[0]
    n = 0
    for blk in f.blocks:
        out_insts = []
        for inst in blk.instructions:
            si = inst.sync_info
            if (
                si is not None
                and len(si.on_wait) > 1
                and type(inst).__name__ in _SPLIT_TYPES
            ):
                waits = list(si.on_wait)
                for w in waits[:-1]:
                    d = mybir.InstDrain(name=f"waitfix{n}", ins=[], outs=[])
                    d.engine = inst.engine
                    d.sync_info = mybir.SyncInfo(on_wait=[w], on_update=[])
                    out_insts.append(d)
                    n += 1
                inst.sync_info = mybir.SyncInfo(
                    on_wait=[waits[-1]], on_update=list(si.on_update)
                )
            out_insts.append(inst)
        blk.instructions = out_insts


_NC_CACHE = {}


def _get_nc(key=()):
    if key not in _NC_CACHE:
        _NC_CACHE[key] = build_nc()
    return _NC_CACHE[key]


def _quant_w_fp8(Wp):
    """Per-row scale search: minimize fp8 rounding error of each row.
    Returns (raw fp8 values of Wp/s_row, s_row)."""
    rowmax = np.abs(Wp).max(axis=1, keepdims=True)
    cands = np.geomspace(0.55, 1.05, 24)
    best_q = np.empty(Wp.shape, dtype=F8)
    best_e = np.full(Wp.shape[0], np.inf)
    best_s = np.empty((Wp.shape[0], 1), np.float32)
    for c in cands:
        s_row = (rowmax / (240.0 * c)).astype(np.float32)
        q = np.clip(Wp / s_row, -240, 240).astype(F8)
        e2 = ((q.astype(np.float32) * s_row - Wp) ** 2).sum(axis=1)
        take = e2 < best_e
        best_q[take] = q[take]
        best_e[take] = e2[take]
        best_s[take] = s_row[take]
    return best_q, best_s[:, 0]


def make_in_maps(x, qweight, scales, zeros, bias):
    x2 = np.asarray(x).reshape(TOK, IN)
    qweight = np.asarray(qweight)
    scales = np.asarray(scales).astype(np.float32)
    zeros = np.asarray(zeros).astype(np.float32)

    # ---- dequantize W on host ----
    nib = np.empty((OUT, IN), np.float32)
    nib[:, 0::2] = (qweight & 15).astype(np.float32)
    nib[:, 1::2] = ((qweight >> 4) & 15).astype(np.float32)
    W = (nib.reshape(OUT, NK, GROUP) - zeros[:, :, None]) * scales[:, :, None]
    W = W.reshape(OUT, IN)

    # ---- fp8 part: group-mean removal + per-row scale search ----
    Wf = W[:, :KF].reshape(OUT, NF, GROUP)
    m = Wf.mean(axis=2)                                   # [OUT, NF]
    Wp = (Wf - m[:, :, None]).reshape(OUT, KF)
    Wq8, s_row = _quant_w_fp8(Wp)                         # raw fp8, [OUT]

    inv_s = (1.0 / s_row)[:, None].astype(np.float32)
    Wb = (W[:, KF:] * inv_s).astype(np.float16)           # [OUT, KB] fp16

    # ---- x: fp8 part (DoubleRow pair layout) + fp16 part + virtual ----
    xg_full = x2[:, :KF].reshape(TOK, NF, GROUP).sum(axis=2)  # fp32 group sums

    xt8_shards, xtb_shards = [], []
    for t in range(T_WAYS):
        xs = x2[t * TSH : (t + 1) * TSH]
        xq8 = xs[:, :KF].astype(F8)                       # [TSH, KF] raw fp8
        a8 = np.ascontiguousarray(xq8.T)                  # [KF, TSH]
        a8 = a8.reshape(NPAIR, 2, 128, N_CHUNK, CHUNK).transpose(3, 2, 1, 0, 4)
        xt8_shards.append(
            np.ascontiguousarray(a8.reshape(N_CHUNK, 128, 2, NPAIR * CHUNK))
        )

        xb = xs[:, KF:].astype(np.float16)                # [TSH, KB]
        ab = np.ascontiguousarray(xb.T)
        ab = ab.reshape(NB, 128, N_CHUNK, CHUNK).transpose(2, 1, 0, 3)
        xtb_shards.append(
            np.ascontiguousarray(ab.reshape(N_CHUNK, 128, NBV * CHUNK))
        )

    in_maps = []
    srow_shards = []
    for c in range(N_CORES):
        o0 = (c % O_WAYS) * OSH
        wq8 = Wq8[o0 : o0 + OSH]                          # [OSH, KF] fp8
        w8 = np.ascontiguousarray(wq8.T)                  # [KF, OSH]
        w8 = np.ascontiguousarray(
            w8.reshape(NPAIR, 2, 128, OSH).transpose(0, 2, 1, 3)
        )                                                 # [NPAIR, 128, 2, OSH]

        wtb = np.ascontiguousarray(Wb[o0 : o0 + OSH].T).reshape(NB, 128, OSH)

        in_maps.append(
            {
                "xt8": xt8_shards[c // O_WAYS],
                "xtb": xtb_shards[c // O_WAYS],
                "wt8": w8,
                "wtb": wtb,
            }
        )
        srow_shards.append(s_row[o0 : o0 + OSH])
    return in_maps, srow_shards, xg_full, m


def _run(x, qweight, scales, zeros, bias, trace=False, **kw):
    nc = _get_nc()
    in_maps, srow_shards, xg_full, m = make_in_maps(x, qweight, scales, zeros, bias)
    res = run_bass_kernel_spmd(nc, in_maps, list(range(N_CORES)), trace=trace, **kw)
    full = np.empty((TOK, OUT), dtype=np.float32)
    for c in range(N_CORES):
        o0 = (c % O_WAYS) * OSH
        t0 = (c // O_WAYS) * TSH
        full[t0 : t0 + TSH, o0 : o0 + OSH] = np.asarray(res.results[c]["out"]).astype(
            np.float32
        ) * srow_shards[c][None, :]
    # exact rank-NF group-mean term (removed from W before fp8 quantization)
    full += xg_full @ m.T
    full += np.asarray(bias, dtype=np.float32)[None, :]
    return full.reshape(B, S, OUT), res


def kernel(x, qweight, scales, zeros, bias):
    out, _ = _run(x, qweight, scales, zeros, bias)
    return out
